# revision 11
# baseline (speedup 1.0000x reference)
"""CrossScaleAttention (GNN segment-softmax attention) on 8 TRN2 NeuronCores.

Math refactor (all FLOPs on device, host only re-lays-out raw inputs):
  score[e] = Q[dst_e] . K[src_e],  Q = dst @ Wq^T + bq,  K = src @ Wk^T + bk
           = Q'[dst_e] . src_feat[src_e] + Q[dst_e].bk   (Q' = Q @ Wk)
  The per-dst constant Q[d].bk cancels in the segment softmax, so only
  Q' (a small per-dst table, computed on device) and raw src_feat rows are
  needed per edge.  Likewise the V projection commutes with the attention-
  weighted sum: out[d] = (sum_e attn_e * src_feat[src_e]) @ Wv^T + bv.

Sharding: dst nodes range-partitioned across 8 cores; edges sorted by dst on
host and laid out edge-major (128-edge tiles x 128-dst blocks, padded to a
uniform tile count per block).  The host ships src_feat rows in both [e, fi]
and [fi, e] tile orientations so the device streams them contiguously at full
DMA bandwidth (no indirect DMA).  Per tile the device computes all-pairs
scores (one matmul), exps them (ACT, batched 4 tiles per PSUM bank), masks
them by multiplying with a device-built (iota == dloc) indicator (DVE/Pool),
and accumulates the weighted segment sums (one matmul into PSUM).  Padding
slots die via the indicator (dloc = -1), so no mask stream is needed.
"""
import sys
sys.path.insert(0, "/opt/trn_rl_repo")

import numpy as np

import concourse.bass as bass
import concourse.bacc as bacc
import concourse.tile as tile
import concourse.mybir as mybir

N_NODES = 50000
D = 128
N_CORES = 8
NDST_CORE = N_NODES // N_CORES          # 6250
DBLK = 128
NBLK = (NDST_CORE + DBLK - 1) // DBLK   # 49
NDST_PAD = NBLK * DBLK                  # 6272
SCALE = 4.0

F32 = mybir.dt.float32
F16 = mybir.dt.float16
BF16 = mybir.dt.bfloat16

_cache = {}


def _build_program(t_blk, reps, ablate=frozenset()):
    """One SPMD program for all 8 cores. t_blk = tiles per dst block."""
    n_tiles = NBLK * t_blk

    nc = bacc.Bacc("TRN2", target_bir_lowering=False, debug=False,
                   enable_asserts=True, num_devices=N_CORES)
    dram = {}

    def din(name, shape, dt):
        dram[name] = nc.dram_tensor(name, shape, dt, kind="ExternalInput").ap()
        return dram[name]

    t_esrcT = din("esrcT", [NBLK, 128, t_blk * 128], F16)
    t_esrcA = din("esrcA", [NBLK, 128, t_blk * 129], F16)
    t_dstloc = din("dstloc", [128, n_tiles], F32)
    t_dstT = din("dstT", [128, NDST_PAD], F16)
    t_wqT = din("WqT", [128, 128], F16)
    t_wk = din("Wk", [128, 128], F16)
    t_wvT = din("WvT", [128, 128], F16)
    t_bq = din("bq", [128, 1], F32)
    t_bv = din("bv", [128, 1], F32)
    t_iota16 = din("iota16", [128, 128], F16)
    t_ident16 = din("ident16", [128, 128], F16)
    t_out = nc.dram_tensor("outT", [128, NDST_PAD], F32,
                           kind="ExternalOutput").ap()

    with tile.TileContext(nc) as tc:
        with tc.tile_pool(name="consts", bufs=1) as cpool, \
             tc.tile_pool(name="qp", bufs=1) as qpool, \
             tc.tile_pool(name="stream", bufs=3) as spool, \
             tc.tile_pool(name="work", bufs=3) as wpool, \
             tc.tile_pool(name="blk", bufs=2) as bpool, \
             tc.tile_pool(name="sc", bufs=2, space="PSUM") as scpool, \
             tc.tile_pool(name="psagg", bufs=2, space="PSUM") as pagpool, \
             tc.tile_pool(name="psmisc", bufs=1, space="PSUM") as pmpool, \
             tc.tile_pool(name="psq", bufs=1, space="PSUM") as pqpool:

            # ---- constants ----
            wqT = cpool.tile([128, 128], F16)
            nc.sync.dma_start(wqT[:], t_wqT[:])
            wk = cpool.tile([128, 128], F16)
            nc.sync.dma_start(wk[:], t_wk[:])
            wvT = cpool.tile([128, 128], F16)
            nc.sync.dma_start(wvT[:], t_wvT[:])
            bq = cpool.tile([128, 1], F32)
            nc.sync.dma_start(bq[:], t_bq[:])
            bv = cpool.tile([128, 1], F32)
            nc.sync.dma_start(bv[:], t_bv[:])
            iota16 = cpool.tile([128, 128], F16)
            nc.sync.dma_start(iota16[:], t_iota16[:])
            ident16 = cpool.tile([128, 128], F16)
            nc.sync.dma_start(ident16[:], t_ident16[:])
            dstT = cpool.tile([128, NDST_PAD], F16)
            nc.sync.dma_start(dstT[:], t_dstT[:])
            dstloc = cpool.tile([128, n_tiles], F32)
            nc.sync.dma_start(dstloc[:], t_dstloc[:])

            qpT = qpool.tile([128, NDST_PAD], F16)     # Q'^T, f16

            # P2 split in halves: qt (Wq matmul + bias) at block start, qp
            # (Wk matmul + qpT store) at block end — so the PE queue never
            # stalls waiting on the cross-engine bias-add.
            def p2_qt(b):
                qt_ps = pqpool.tile([128, 128], F32, tag="qt")
                nc.tensor.matmul(qt_ps[:], lhsT=wqT[:],
                                 rhs=dstT[:, b * 128:(b + 1) * 128],
                                 start=True, stop=True)
                qt_sb = wpool.tile([128, 128], F16, tag="qtsb")
                nc.vector.tensor_scalar(
                    out=qt_sb[:], in0=qt_ps[:], scalar1=bq[:, :1],
                    scalar2=None, op0=mybir.AluOpType.add)
                return qt_sb

            def p2_qp(b, qt_sb):
                qp_ps = pqpool.tile([128, 128], F32, tag="qp")
                nc.tensor.matmul(qp_ps[:], lhsT=wk[:], rhs=qt_sb[:],
                                 start=True, stop=True)
                nc.scalar.activation(qpT[:, b * 128:(b + 1) * 128], qp_ps[:],
                                     mybir.ActivationFunctionType.Copy)

            def body(_iv=None):
                # prologue: Q' for blocks 0,1 (edge loop runs 2 blocks behind)
                for b in (0, 1):
                    p2_qp(b, p2_qt(b))

                # ---- P4: edge phase, P5 fused at block end ----
                qt_pend = None
                for b in range(NBLK):
                    eT = spool.tile([128, t_blk * 128], F16, tag="eT")
                    eA = spool.tile([128, t_blk * 129], F16, tag="eA")
                    nc.sync.dma_start(eT[:], t_esrcT[b])
                    nc.scalar.dma_start(eA[:], t_esrcA[b])
                    if b + 2 < NBLK:
                        qt_pend = p2_qt(b + 2)
                    aggP = pagpool.tile([128, 129], F32, tag="agg")
                    for g in range(0, t_blk, 4):
                        gw = min(4, t_blk - g)
                        sc = scpool.tile([128, 512], F32, tag="sc")
                        for j in range(gw):
                            t = g + j
                            nc.tensor.matmul(
                                sc[:, j * 128:(j + 1) * 128],
                                lhsT=eT[:, t * 128:(t + 1) * 128],
                                rhs=qpT[:, b * 128:(b + 1) * 128],
                                start=True, stop=True)
                        E4 = wpool.tile([128, 512], BF16, tag="E")
                        nc.scalar.activation(E4[:, :gw * 128], sc[:, :gw * 128],
                                             mybir.ActivationFunctionType.Exp,
                                             scale=1.0 / SCALE)
                        eq4 = wpool.tile([128, 512], F16, tag="eq")
                        for j in range(gw):
                            t = g + j
                            gt = b * t_blk + t
                            eng = nc.vector if gt % 2 == 0 else nc.gpsimd
                            eng.tensor_scalar(
                                out=eq4[:, j * 128:(j + 1) * 128],
                                in0=iota16[:], scalar1=dstloc[:, gt:gt + 1],
                                scalar2=None, op0=mybir.AluOpType.is_equal)
                        Ep4 = wpool.tile([128, 512], BF16, tag="Ep")
                        nc.vector.tensor_tensor(
                            out=Ep4[:, :gw * 128], in0=E4[:, :gw * 128],
                            in1=eq4[:, :gw * 128], op=mybir.AluOpType.mult)
                        for j in range(gw):
                            t = g + j
                            nc.tensor.matmul(
                                aggP[:],
                                lhsT=Ep4[:, j * 128:(j + 1) * 128],
                                rhs=eA[:, t * 129:t * 129 + 129],
                                start=(t == 0), stop=(t == t_blk - 1))
                    if b + 2 < NBLK:
                        p2_qp(b + 2, qt_pend)
                    # ---- block end: divide, transpose, V-project, emit ----
                    dn = bpool.tile([128, 1], F32, tag="dn")
                    nc.vector.tensor_scalar(
                        out=dn[:], in0=aggP[:, 128:129], scalar1=1e-30,
                        scalar2=None, op0=mybir.AluOpType.max)
                    rc = bpool.tile([128, 1], F32, tag="rc")
                    nc.vector.reciprocal(rc[:], dn[:])
                    aggN = bpool.tile([128, 128], F16, tag="aggN")
                    nc.vector.tensor_scalar(
                        out=aggN[:], in0=aggP[:, :128], scalar1=rc[:, :1],
                        scalar2=None, op0=mybir.AluOpType.mult)
                    tr_ps = pmpool.tile([128, 128], F16, tag="mmt")
                    nc.tensor.transpose(tr_ps[:], aggN[:], ident16[:])
                    aggT = bpool.tile([128, 128], F16, tag="aggT")
                    nc.vector.tensor_scalar(
                        out=aggT[:], in0=tr_ps[:], scalar1=0.0,
                        scalar2=None, op0=mybir.AluOpType.add)
                    o_ps = pmpool.tile([128, 128], F32, tag="mm")
                    nc.tensor.matmul(o_ps[:], lhsT=wvT[:], rhs=aggT[:],
                                     start=True, stop=True)
                    o_sb = bpool.tile([128, 128], F32, tag="osb")
                    nc.vector.tensor_scalar(
                        out=o_sb[:], in0=o_ps[:], scalar1=bv[:, :1],
                        scalar2=None, op0=mybir.AluOpType.add)
                    nc.gpsimd.dma_start(t_out[:, b * 128:(b + 1) * 128], o_sb[:])

            if reps == 1:
                body()
            else:
                with tc.For_i(0, reps, 1):
                    body()

    nc.compile()
    return nc


def _prep(src_feat, dst_feat, src_idx, dst_idx, Wq, bq, Wk, bk, Wv, bv):
    """Host-side layout: sort edges by dst, shard by dst range, build tiles."""
    src_feat = np.asarray(src_feat, np.float32)
    dst_feat = np.asarray(dst_feat, np.float32)
    src_idx = np.asarray(src_idx).astype(np.int64)
    dst_idx = np.asarray(dst_idx).astype(np.int64)

    order = np.argsort(dst_idx, kind="stable")
    d_sorted = dst_idx[order]
    s_sorted = src_idx[order]

    core_lo = np.searchsorted(d_sorted, np.arange(N_CORES) * NDST_CORE)
    core_hi = np.searchsorted(d_sorted, (np.arange(N_CORES) + 1) * NDST_CORE)

    # per (core, block) edge counts -> global uniform t_blk
    blk_of_edge = (d_sorted % NDST_CORE) // DBLK  # valid within a core's range
    t_blk = 0
    counts = []
    for c in range(N_CORES):
        cnt = np.bincount(blk_of_edge[core_lo[c]:core_hi[c]], minlength=NBLK)
        counts.append(cnt)
        t_blk = max(t_blk, int(np.ceil(cnt.max() / 128)))
    t_blk = t_blk + (t_blk % 2)  # even
    n_tiles = NBLK * t_blk
    n_slots = n_tiles * 128

    src16 = src_feat.astype(np.float16)
    in_maps = []
    for c in range(N_CORES):
        lo, hi = core_lo[c], core_hi[c]
        s_c = s_sorted[lo:hi]
        dloc_c = (d_sorted[lo:hi] % NDST_CORE) % DBLK
        blk_c = blk_of_edge[lo:hi]
        cnt = counts[c]
        # slot index for each edge: block base + position within block
        off_in_blk = np.arange(hi - lo) - np.repeat(
            np.concatenate([[0], np.cumsum(cnt)[:-1]]), cnt)
        slot = blk_c * (t_blk * 128) + off_in_blk

        srcslot = np.zeros(n_slots, np.int64)
        dlocslot = np.full(n_slots, -1.0, np.float32)
        srcslot[slot] = s_c
        dlocslot[slot] = dloc_c.astype(np.float32)

        esrc = src16[srcslot]                                # [n_slots, 128]
        esrc = esrc.reshape(n_tiles, 128, 128)
        # esrcA: [NBLK, 128, t_blk*129], per-tile [e, fi] + ones column
        eA = np.empty((n_tiles, 128, 129), np.float16)
        eA[:, :, :128] = esrc
        eA[:, :, 128] = 1.0
        eA = eA.reshape(NBLK, t_blk, 128, 129).transpose(0, 2, 1, 3)
        eA = np.ascontiguousarray(eA).reshape(NBLK, 128, t_blk * 129)
        # esrcT: per-tile transpose [fi, e] -> [NBLK, 128, t_blk*128]
        eT = esrc.transpose(0, 2, 1).reshape(NBLK, t_blk, 128, 128)
        eT = np.ascontiguousarray(eT.transpose(0, 2, 1, 3)).reshape(
            NBLK, 128, t_blk * 128)

        dstloc = np.ascontiguousarray(
            dlocslot.reshape(n_tiles, 128).T)                # [128, n_tiles]

        dT = np.zeros((128, NDST_PAD), np.float16)
        dT[:, :NDST_CORE] = dst_feat[c * NDST_CORE:(c + 1) * NDST_CORE].T

        in_maps.append({
            "esrcT": eT, "esrcA": eA, "dstloc": dstloc, "dstT": dT,
            "WqT": np.ascontiguousarray(np.asarray(Wq, np.float16).T),
            "Wk": np.ascontiguousarray(np.asarray(Wk, np.float16)),
            "WvT": np.ascontiguousarray(np.asarray(Wv, np.float16).T),
            "bq": np.asarray(bq, np.float32).reshape(128, 1),
            "bv": np.asarray(bv, np.float32).reshape(128, 1),
            "iota16": np.tile(np.arange(128, dtype=np.float16), (128, 1)),
            "ident16": np.eye(128, dtype=np.float16),
        })
    return in_maps, t_blk, dst_idx


def _run(nc, in_maps):
    from concourse.bass_utils import run_bass_kernel_spmd
    res = run_bass_kernel_spmd(nc, in_maps, list(range(N_CORES)))
    return res.results


def kernel(src_feat, dst_feat, src_idx, dst_idx, Wq, bq, Wk, bk, Wv, bv):
    in_maps, t_blk, dst_idx_np = _prep(src_feat, dst_feat, src_idx, dst_idx,
                                       Wq, bq, Wk, bk, Wv, bv)
    key = (t_blk, 1)
    if key not in _cache:
        _cache[key] = _build_program(t_blk, 1)
    nc = _cache[key]
    results = _run(nc, in_maps)

    out = np.empty((N_NODES, D), np.float32)
    for c in range(N_CORES):
        out[c * NDST_CORE:(c + 1) * NDST_CORE] = \
            results[c]["outT"][:, :NDST_CORE].T
    # degree-0 dst rows: reference yields 0, device yields bv — fix up
    deg = np.bincount(dst_idx_np, minlength=N_NODES)
    if (deg == 0).any():
        out[deg == 0] = 0.0
    return out
